# revision 10
# baseline (speedup 1.0000x reference)
"""Bidirectional LSTM (TF BasicLSTMCell semantics) on 8 Trainium2 NeuronCores.

Problem: x [64, 128, 512], per-direction W [1024, 2048], b [2048].
out [64, 128, 1024] = concat(h_fw, h_bw) over a T=128 sequential scan.

Sharding: 2 (direction) x 4 (batch quarters) = 8 cores, B_local = 16.
Every core runs the SAME program; direction is handled host-side by
time-reversing x (and the returned outputs) for the backward cores and
binding W_bw instead of W_fw.

Per-core program (v2):
  phase 1:  G^T = Wx^T @ x^T + b (+1 on the f gate) -- fp16 matmuls over all
            T*B columns in 512-col groups.  Group cc=0 runs ahead of the
            scan; the remaining 48 groups are interleaved one per scan step
            so they fill tensor-engine gaps in the recurrence.
  scan:     per step and per hidden half (256 dims = 2 PE column tiles):
            one PSUM tile [128, 8 zones x 16] holds all four gates; G(t) is
            preloaded into PSUM by an identity matmul (start=True), then 32
            fp16 recurrent matmuls accumulate Whh^T h.  Gate zones are
            ordered (i, f, o, j) so one Sigmoid covers i|f|o and one Tanh
            covers j, both reading PSUM directly -- no vector-engine z-adds.
            c update is 3 DVE ops; h is produced once in fp16 (recurrence)
            and once in fp32 on GpSimd (output staging, off critical path).
  output:   h stored hidden-major [512, T*B] fp32 in SBUF, DMA'd out in
            32-step chunks during the scan; host transposes in numpy.
"""

import os
import sys

import numpy as np

for _p in ("/opt/trn_rl_repo", "/root/.axon_site/_ro/trn_rl_repo"):
    if os.path.isdir(_p) and _p not in sys.path:
        sys.path.insert(0, _p)

from contextlib import ExitStack

import concourse.bass as bass
import concourse.mybir as mybir
import concourse.tile as tile
from concourse import bacc

F32 = mybir.dt.float32
F16 = mybir.dt.float16
AF = mybir.ActivationFunctionType

B_FULL = 64
B_LOC = 16  # batch per core
T = 128
F = 512
H = 512
NG = 4 * H  # 2048 gate columns
KT = 4      # 128-row contraction tiles over F or H
MT = 16     # 128-col gate tiles
FORGET_BIAS = 1.0

# Gate-tile storage order: for each hidden half, zones (i0,i1,f0,f1,o0,o1,j0,j1)
# where the suffix is the 128-dim quarter within the half.  Original W gate
# order is (i, j, f, o).
ZONE_GATES = (0, 2, 3, 1)  # zone gate order i,f,o,j -> original gate index
PERM = [4 * g + 2 * hh + mr for hh in (0, 1) for g in ZONE_GATES for mr in (0, 1)]


def build_nc(t_steps: int = T, repeat: int = 1) -> bass.Bass:
    tb = t_steps * B_LOC

    nc = bacc.Bacc("TRN2", target_bir_lowering=False, debug=False)
    x_d = nc.dram_tensor("xT", [F, tb], F16, kind="ExternalInput").ap()
    wx_d = nc.dram_tensor("wx", [F, NG], F16, kind="ExternalInput").ap()
    whh_d = nc.dram_tensor("whh", [H, NG], F16, kind="ExternalInput").ap()
    bias_d = nc.dram_tensor("bias", [128, MT], F32, kind="ExternalInput").ap()
    id_d = nc.dram_tensor("ident", [128, 128], F16, kind="ExternalInput").ap()
    y_d = nc.dram_tensor("y", [H, tb], F32, kind="ExternalOutput").ap()

    with ExitStack() as ctx:
        tc = ctx.enter_context(tile.TileContext(nc))
        const = ctx.enter_context(tc.tile_pool(name="const", bufs=1))
        wx_sb = const.tile([128, KT * NG], F16, tag="wx")    # col = k*NG + m*128 + j
        whh_sb = const.tile([128, KT * NG], F16, tag="whh")  # col = k*NG + m*128 + j
        xT_sb = const.tile([128, KT * tb], F16, tag="xT")    # col = k*tb + (t,b)
        g_sb = const.tile([128, 2 * t_steps * 128], F16, tag="g")  # col = hh*(T*128) + t*128 + z*16 + b
        hall = const.tile([128, KT * tb], F32, tag="hall")   # col = k*tb + t*16 + b
        bias_sb = const.tile([128, MT], F32, tag="bias")
        id_sb = const.tile([128, 128], F16, tag="ident")
        c_sb = const.tile([128, 2 * 2 * B_LOC], F32, tag="c")  # col = hh*32 + mr*16 + b

        for k in range(KT):
            nc.sync.dma_start(wx_sb[:, k * NG:(k + 1) * NG], wx_d[k * 128:(k + 1) * 128, :])
            nc.sync.dma_start(whh_sb[:, k * NG:(k + 1) * NG], whh_d[k * 128:(k + 1) * 128, :])
            nc.sync.dma_start(xT_sb[:, k * tb:(k + 1) * tb], x_d[k * 128:(k + 1) * 128, :])
        nc.sync.dma_start(bias_sb[:], bias_d[:, :])
        nc.sync.dma_start(id_sb[:], id_d[:, :])

        if repeat > 1:
            loop_cm = tc.For_i(0, repeat, 1)
            loop_cm.__enter__()

        nc.vector.memset(c_sb[:], 0.0)

        g4 = g_sb[:].rearrange("p (h t z b) -> p h t z b", h=2, t=t_steps, z=8)
        g2 = g_sb[:].rearrange("p (h c) -> p h c", h=2)
        h3 = hall[:].rearrange("p (k c) -> p k c", k=KT)

        w1 = min(512, tb)
        ncc = tb // w1
        tpc = w1 // B_LOC  # timesteps per phase-1 column group

        with tc.tile_pool(name="p1", bufs=3, space="PSUM") as p1_p, \
             tc.tile_pool(name="sps", bufs=2, space="PSUM") as sps_p, \
             tc.tile_pool(name="act", bufs=3) as a_p, \
             tc.tile_pool(name="h16", bufs=3) as h_p, \
             tc.tile_pool(name="tmp", bufs=3) as tmp_p:

            def emit_p1(cc, m):
                ps = p1_p.tile([128, w1], F32, tag="p1", name="p1")
                for k in range(KT):
                    nc.tensor.matmul(
                        ps[:],
                        wx_sb[:, k * NG + m * 128: k * NG + (m + 1) * 128],
                        xT_sb[:, k * tb + cc * w1: k * tb + (cc + 1) * w1],
                        start=(k == 0), stop=(k == KT - 1),
                    )
                hh, z = divmod(m, 8)
                dest = g4[:, hh, cc * tpc:(cc + 1) * tpc, z, :]
                # GpSimd cannot read PSUM on hardware; Identity-activation
                # with a per-partition bias does the add+downcast on Act
                nc.scalar.add(
                    dest,
                    ps[:].rearrange("p (t b) -> p t b", t=tpc),
                    bias_sb[:, m:m + 1],
                )

            for m in range(MT):
                emit_p1(0, m)
            p1_queue = [(cc, m) for cc in range(1, ncc) for m in range(MT)]

            hq = [None] * KT
            for hh in (0, 1):
                t0 = h_p.tile([128, 2 * B_LOC], F16, tag="h16_%d" % hh,
                              name="h0_%d" % hh)
                nc.vector.memset(t0[:], 0.0)
                hq[2 * hh] = t0[:, 0:B_LOC]
                hq[2 * hh + 1] = t0[:, B_LOC:2 * B_LOC]

            for t in range(t_steps):
                ts_ = slice(t * B_LOC, (t + 1) * B_LOC)
                # --- PE: G(t) preload + 32 recurrent matmuls per half.
                # k=0,1 blocks (ready when half0's h lands) are issued for
                # BOTH halves before any k=2,3 block so that after half1's h
                # arrives only the k=2,3 tail remains in front of the psums.
                pss = {}
                for hh in (0, 1):
                    ps = sps_p.tile([128, 8 * B_LOC], F32, tag="ps%d" % hh,
                                    name="ps%d" % hh)
                    pss[hh] = ps
                    nc.tensor.matmul(
                        ps[:], id_sb[:], g2[:, hh, t * 128:(t + 1) * 128],
                        start=True, stop=False, skip_group_check=True)
                for ks in ((0, 1), (2, 3)):
                    for hh in (0, 1):
                        for k in ks:
                            for z in range(8):
                                m = hh * 8 + z
                                nc.tensor.matmul(
                                    pss[hh][:, z * B_LOC:(z + 1) * B_LOC],
                                    whh_sb[:, k * NG + m * 128: k * NG + (m + 1) * 128],
                                    hq[k],
                                    start=False, stop=(k == KT - 1 and z == 7),
                                    skip_group_check=True)
                # fill PE gaps with one deferred phase-1 group per step
                if p1_queue:
                    emit_p1(*p1_queue.pop(0))
                # --- Act: one sigmoid over all four gate zones per half;
                # tanh(z_j) = 2*sigmoid(2 z_j) - 1 with the 2x host-folded
                # into the j-gate weights/bias, reconstructed on DVE.
                sio = {}
                for hh in (0, 1):
                    s = a_p.tile([128, 8 * B_LOC], F32, tag="sio%d" % hh,
                                 name="sio%d" % hh)
                    nc.scalar.activation(s[:], pss[hh][:], AF.Sigmoid)
                    sio[hh] = s
                # --- DVE: c update per half
                for hh in (0, 1):
                    ch = c_sb[:, hh * 32:(hh + 1) * 32]
                    tj = tmp_p.tile([128, 2 * B_LOC], F32, tag="tj%d" % hh,
                                    name="tj%d" % hh)
                    nc.vector.tensor_scalar(
                        tj[:], sio[hh][:, 96:128], 2.0, -1.0,
                        mybir.AluOpType.mult, mybir.AluOpType.add)
                    nc.vector.tensor_mul(ch, ch, sio[hh][:, 32:64])
                    tmp = tmp_p.tile([128, 2 * B_LOC], F32, tag="tmp%d" % hh,
                                     name="tmp%d" % hh)
                    nc.vector.tensor_mul(tmp[:], sio[hh][:, 0:32], tj[:])
                    nc.vector.tensor_add(ch, ch, tmp[:])
                # --- Act: tanh(c)
                tanc = {}
                for hh in (0, 1):
                    tct = a_p.tile([128, 2 * B_LOC], F32, tag="tanc%d" % hh,
                                   name="tanc%d" % hh)
                    nc.scalar.activation(tct[:], c_sb[:, hh * 32:(hh + 1) * 32], AF.Tanh)
                    tanc[hh] = tct
                # --- h: fp16 for the recurrence (DVE), fp32 staging (GpSimd)
                for hh in (0, 1):
                    hnew = h_p.tile([128, 2 * B_LOC], F16, tag="h16_%d" % hh,
                                    name="hn%d" % hh)
                    nc.vector.tensor_mul(hnew[:], tanc[hh][:], sio[hh][:, 64:96])
                    hq[2 * hh] = hnew[:, 0:B_LOC]
                    hq[2 * hh + 1] = hnew[:, B_LOC:2 * B_LOC]
                    nc.gpsimd.tensor_mul(
                        h3[:, 2 * hh:2 * hh + 2, ts_],
                        tanc[hh][:].rearrange("p (m c) -> p m c", m=2),
                        sio[hh][:, 64:96].rearrange("p (m c) -> p m c", m=2),
                    )
                # --- chunked output DMA during the scan
                if (t + 1) % tpc == 0:
                    ci = t // tpc
                    for k in range(KT):
                        nc.sync.dma_start(
                            y_d[k * 128:(k + 1) * 128, ci * w1:(ci + 1) * w1],
                            hall[:, k * tb + ci * w1: k * tb + (ci + 1) * w1])

        if repeat > 1:
            loop_cm.__exit__(None, None, None)

    nc.compile()
    return nc


_BUILT: bass.Bass | None = None


def _get_built() -> bass.Bass:
    global _BUILT
    if _BUILT is None:
        _BUILT = build_nc(T)
    return _BUILT


def make_in_maps(x, W_fw, b_fw, W_bw, b_bw, t_steps: int = T):
    x = np.asarray(x, np.float32)
    colperm = np.concatenate([np.arange(m * 128, (m + 1) * 128) for m in PERM])
    ident = np.eye(128, dtype=np.float16)
    in_maps = []
    for d, (Wd, bd) in enumerate(((W_fw, b_fw), (W_bw, b_bw))):
        Wd = np.asarray(Wd, np.float32).copy()
        bv = np.asarray(bd, np.float32).copy()
        # tanh(z_j) is computed on-device as 2*sigmoid(2 z_j) - 1; fold the 2x
        # into the j-gate weights/bias so all four gates share one Sigmoid.
        Wd[:, H:2 * H] *= 2.0
        bv[H:2 * H] *= 2.0
        bv[2 * H:3 * H] += FORGET_BIAS  # fold forget bias into the f-gate bias
        wx = np.ascontiguousarray(Wd[:F][:, colperm]).astype(np.float16)
        whh = np.ascontiguousarray(Wd[F:][:, colperm]).astype(np.float16)
        bias = np.ascontiguousarray(bv[colperm].reshape(MT, 128).T)
        for g in range(4):
            xg = x[g * B_LOC:(g + 1) * B_LOC, :t_steps]
            if d == 1:
                xg = xg[:, ::-1, :]
            x_t = np.ascontiguousarray(
                xg.transpose(1, 0, 2).reshape(t_steps * B_LOC, F).T
            ).astype(np.float16)
            in_maps.append({"xT": x_t, "wx": wx, "whh": whh, "bias": bias,
                            "ident": ident})
    return in_maps


def assemble_out(results, t_steps: int = T):
    out = np.empty((B_FULL, t_steps, 2 * H), np.float32)
    for idx, r in enumerate(results):
        d, g = divmod(idx, 4)
        h = r["y"].reshape(H, t_steps, B_LOC).transpose(2, 1, 0)  # [16, T, 512]
        if d == 1:
            h = h[:, ::-1, :]
        out[g * B_LOC:(g + 1) * B_LOC, :, d * H:(d + 1) * H] = h
    return out


def kernel(x, W_fw, b_fw, W_bw, b_bw):
    from concourse.bass_utils import run_bass_kernel_spmd

    nc = _get_built()
    in_maps = make_in_maps(x, W_fw, b_fw, W_bw, b_bw)
    res = run_bass_kernel_spmd(nc, in_maps, core_ids=list(range(8)))
    return assemble_out(res.results)
